# revision 10
# baseline (speedup 1.0000x reference)
"""Distributed causal attention head for TRN2 (8 NeuronCores), v2.

Problem: B=4, S=4096, D=1024, H=64 fp32.
  q,k,v = x @ W{q,k,v}; scores = q k^T / sqrt(H); causal softmax; out = P v.

Design (collective-free, one SPMD-uniform NEFF, no barrier/AllGather/RS):
  - 4 batches x 2 cores per batch. Each core receives the FULL batch x^T
    (bf16, host-pretransposed) and projects Q^T for ALL 4096 queries
    itself; K/V only for the 2048 interleaved key rows it owns
    (128-chunk interleave keeps the causal work perfectly balanced).
  - SPMD uniformity: the host permutes x^T columns per core so the
    core's OWN key chunks sit at even 128-chunk positions. All extraction
    addresses are then identical across cores; causality differences are
    absorbed into per-core 0/1 mask DATA (queries are consistently in the
    permuted order; the host un-permutes the output).
  - Projections run dc-outer (stationary [Wq|Wk] loaded once per dc
    chunk, 4 moving blocks of 512) to amortize LDWEIGHTS; V is computed
    NATURALLY ([keys,64], x^T chunk stationary, Wv moving) so no PE
    transposes are needed.
  - Attention is emitted as two lagged streams so the Tensor engine never
    waits on the Scalar engine (PE DVFS: full 2.4 GHz only after ~3us of
    continuous execution): score matmuls fill [128,1024] PSUM tiles
    (chunk-PAIRS), the Scalar engine exps them into a big persistent
    SBUF P buffer (72KB/partition), and AV matmuls consume P two slots
    behind. V is augmented with a ones column so AV also emits the
    softmax denominator.
  - Per-core partial (num^T | den) = [65, 4096] f32 goes straight to
    DRAM; the HOST adds the two partials of each pair, divides, and
    transposes. No on-device collective at all.
"""

import sys

sys.path.insert(0, "/opt/trn_rl_repo")

import numpy as np
import ml_dtypes

B, S, D, H = 4, 4096, 1024, 64
RPC = S // 2            # key rows owned per core
QB = 512                # query block width
NQB = S // QB           # 8 query blocks
NKC = RPC // 128        # 16 local key chunks
BF16 = ml_dtypes.bfloat16

_CACHE = {}


def _build():
    import concourse.bass as bass
    import concourse.mybir as mybir
    from concourse import bacc, tile
    from concourse.bass import ts

    f32 = mybir.dt.float32
    bf16 = mybir.dt.bfloat16
    Alu = mybir.AluOpType
    Act = mybir.ActivationFunctionType

    nc = bacc.Bacc(None, target_bir_lowering=False)

    xt_ext = nc.declare_dram_parameter("xt", [D, S], bf16, isOutput=False)
    wqk_ext = nc.declare_dram_parameter("wqk", [D, 128], bf16, isOutput=False)
    wv_ext = nc.declare_dram_parameter("wv", [D, H], bf16, isOutput=False)
    mask_ext = nc.declare_dram_parameter("mask", [128, 1024], bf16, isOutput=False)
    out_ext = nc.declare_dram_parameter("out", [H + 1, S], f32, isOutput=True)

    with tile.TileContext(nc) as tc:
        with tc.tile_pool(name="persist", bufs=1) as persist:
            # --- persistent SBUF tensors ---
            wqk_sb = persist.tile([128, 8, 128], bf16, tag="wqk")
            wv_sb = persist.tile([128, 8, H], bf16, tag="wv")
            mask_sb = persist.tile([128, 1024], bf16, tag="mask")
            qT = persist.tile([64, S], bf16, tag="qT")
            kT = persist.tile([64, RPC], bf16, tag="kT")
            v_all = persist.tile([128, NKC, H + 1], bf16, tag="v_all")
            p_sb = persist.tile([128, 36, 1024], bf16, tag="p")

            nc.vector.memset(v_all[:, :, H], 1.0)

            for dc in range(8):
                nc.sync.dma_start(out=wqk_sb[:, dc, :], in_=wqk_ext[ts(dc, 128), :])
                nc.sync.dma_start(out=wv_sb[:, dc, :], in_=wv_ext[ts(dc, 128), :])
            nc.sync.dma_start(out=mask_sb[:], in_=mask_ext[:])

            # --- phase 1: Q|K projections (dc-outer, half-S at a time) ---
            dma_engines = [nc.sync, nc.scalar]
            with tc.tile_pool(name="xt", bufs=16) as xt_pool:
                # one tile per (half, dc) strip so consumers wait only on
                # the strip they read; spread across all 4 DMA queues
                xts = [[None] * 8 for _ in range(2)]
                for hf in range(2):
                    for dc in range(8):
                        strip = xt_pool.tile(
                            [128, S // 2], bf16, tag="xt", name=f"xt{hf}_{dc}"
                        )
                        xts[hf][dc] = strip
                        dma_engines[dc % 2].dma_start(
                            out=strip[:],
                            in_=xt_ext[ts(dc, 128), ts(hf, S // 2)],
                        )

                with tc.tile_pool(name="pj", bufs=4, space="PSUM") as pj_pool:
                    for hf in range(2):
                        # stationary [Wq|Wk] per dc chunk, 4 moving blocks
                        qkps = [
                            pj_pool.tile([128, QB], f32, tag="qk", name=f"qk{s}")
                            for s in range(4)
                        ]
                        for dc in range(8):
                            for s in range(4):
                                nc.tensor.matmul(
                                    qkps[s][:],
                                    lhsT=wqk_sb[:, dc, :],
                                    rhs=xts[hf][dc][:, ts(s, QB)],
                                    start=(dc == 0),
                                    stop=(dc == 7),
                                    skip_group_check=True,
                                )
                        # extraction (DVE): Q^T all columns; K^T from the two
                        # OWN chunks (even 128-chunk positions) of each block.
                        for s in range(4):
                            sg = 4 * hf + s
                            nc.vector.tensor_copy(qT[:, ts(sg, QB)], qkps[s][0:64, :])
                            nc.vector.tensor_copy(
                                kT[:, 256 * sg : 256 * sg + 128],
                                qkps[s][64:128, 0:128],
                            )
                            nc.vector.tensor_copy(
                                kT[:, 256 * sg + 128 : 256 * sg + 256],
                                qkps[s][64:128, 256:384],
                            )

                # --- phase 2: attention with V-projection interleaved ---
                # V chunk pair (2p, 2p+1) is emitted just before diagonal
                # score pair (p, p); AV consumes P two slots behind the
                # score/exp stream so the PE never waits on the Scalar
                # engine's exp.
                pairs = [(t, ip) for t in range(NQB) for ip in range(t + 1)]
                LAG = 2
                av_tiles = {}
                with (
                    tc.tile_pool(name="st", bufs=2, space="PSUM") as st_pool,
                    tc.tile_pool(name="pv", bufs=2, space="PSUM") as pv_pool,
                    tc.tile_pool(name="av", bufs=2, space="PSUM") as av_pool,
                    tc.tile_pool(name="o", bufs=2) as o_pool,
                ):
                    def emit_v_chunk(i):
                        hf, ii = i // 8, i % 8
                        vps = pv_pool.tile([128, H], f32, tag="v", name=f"v{i}")
                        for dc in range(8):
                            nc.tensor.matmul(
                                vps[:],
                                lhsT=xts[hf][dc][:, 256 * ii : 256 * ii + 128],
                                rhs=wv_sb[:, dc, :],
                                start=(dc == 0),
                                stop=(dc == 7),
                            )
                        nc.vector.tensor_copy(v_all[:, i, 0:H], vps[:])

                    for slot in range(len(pairs) + LAG):
                        if slot < len(pairs):
                            t, ip = pairs[slot]
                            if ip == t:  # V chunks needed by av pair (t,t)
                                emit_v_chunk(2 * t)
                                emit_v_chunk(2 * t + 1)
                            st2 = st_pool.tile([128, 1024], f32, tag="st")
                            nc.tensor.matmul(
                                st2[:, 0:512],
                                lhsT=kT[:, 256 * ip : 256 * ip + 128],
                                rhs=qT[:, ts(t, QB)],
                                start=True,
                                stop=True,
                                skip_group_check=True,
                            )
                            nc.tensor.matmul(
                                st2[:, 512:1024],
                                lhsT=kT[:, 256 * ip + 128 : 256 * ip + 256],
                                rhs=qT[:, ts(t, QB)],
                                start=True,
                                stop=True,
                                skip_group_check=True,
                            )
                            nc.scalar.activation(
                                p_sb[:, slot, :], st2[:], Act.Exp, scale=0.125
                            )
                            if ip == t:  # diagonal pair: multiplicative mask
                                nc.vector.tensor_tensor(
                                    p_sb[:, slot, :],
                                    p_sb[:, slot, :],
                                    mask_sb[:],
                                    Alu.mult,
                                )
                        k = slot - LAG
                        if k >= 0:
                            t, ip = pairs[k]
                            if ip == 0:
                                av_tiles[t] = av_pool.tile(
                                    [H + 1, QB], f32, tag="av", name=f"av{t}"
                                )
                            av = av_tiles[t]
                            nc.tensor.matmul(
                                av[:],
                                lhsT=v_all[:, 2 * ip, :],
                                rhs=p_sb[:, k, 0:512],
                                start=(ip == 0),
                                stop=False,
                                skip_group_check=True,
                            )
                            nc.tensor.matmul(
                                av[:],
                                lhsT=v_all[:, 2 * ip + 1, :],
                                rhs=p_sb[:, k, 512:1024],
                                start=False,
                                stop=(ip == t),
                                skip_group_check=True,
                            )
                            if ip == t:
                                o_sb = o_pool.tile([H + 1, QB], f32, tag="o")
                                nc.vector.tensor_copy(o_sb[:], av[:])
                                nc.sync.dma_start(
                                    out=out_ext[:, ts(t, QB)], in_=o_sb[:]
                                )

    nc.finalize()
    return nc


def _make_mask2(g: int) -> np.ndarray:
    """[128, 1024] multiplicative mask for the diagonal chunk pair of any
    query block t (t-independent thanks to the per-core permutation).

    Query columns are in permuted order: position pc in the block maps to
    global query chunk offsets delta = [g, 1-g, 2+g, 3-g] (relative to 4t).
    Left half masks own key chunk at global offset g; right half offset 2+g.
    """
    m = np.zeros((128, 1024), dtype=np.float32)
    delta = [g, 1 - g, 2 + g, 3 - g]
    kk = np.arange(128)[:, None]
    qq = np.arange(128)[None, :]
    for half, keyoff in ((0, g), (1, 2 + g)):
        for pc in range(4):
            keep = (128 * (delta[pc] - keyoff) + qq) >= kk
            m[:, half * 512 + pc * 128 : half * 512 + (pc + 1) * 128] = keep
    return m.astype(BF16)


def _swap_pairs(a: np.ndarray) -> np.ndarray:
    """Swap adjacent 128-column chunks (self-inverse permutation)."""
    n = a.shape[-1]
    return np.ascontiguousarray(
        a.reshape(a.shape[:-1] + (n // 256, 2, 128))[..., ::-1, :].reshape(a.shape)
    )


def _shard_inputs(input, Wq, Wk, Wv):
    wqk = np.ascontiguousarray(np.concatenate([Wq, Wk], axis=1)).astype(BF16)
    wv = np.ascontiguousarray(Wv).astype(BF16)
    masks = [_make_mask2(0), _make_mask2(1)]
    in_maps = []
    for b in range(B):
        xt = np.ascontiguousarray(np.asarray(input)[b].T).astype(BF16)
        xt_sw = _swap_pairs(xt)
        for g in range(2):
            in_maps.append(
                {
                    "xt": xt if g == 0 else xt_sw,
                    "wqk": wqk,
                    "wv": wv,
                    "mask": masks[g],
                }
            )
    return in_maps


def _unshard(results):
    out = np.empty((B, S, H), dtype=np.float32)
    for b in range(B):
        r0 = results[2 * b]["out"]                      # [65, S] natural order
        r1 = _swap_pairs(results[2 * b + 1]["out"])     # un-permute g=1
        m = r0 + r1
        out[b] = (m[:H] / m[H : H + 1]).T
    return out


def _run(inputs, trace=False):
    from concourse.bass_utils import run_bass_kernel_spmd

    if "nc" not in _CACHE:
        _CACHE["nc"] = _build()
    nc = _CACHE["nc"]
    in_maps = _shard_inputs(**inputs)
    res = run_bass_kernel_spmd(nc, in_maps, core_ids=list(range(8)), trace=trace)
    out = _unshard(res.results)
    return out, res


def kernel(**inputs) -> np.ndarray:
    out, _ = _run(inputs, trace=False)
    return out


# revision 11
# speedup vs baseline: 1.2351x; 1.2351x over previous
"""Distributed causal attention head for TRN2 (8 NeuronCores), v4.

Problem: B=4, S=4096, D=1024, H=64 fp32.
  q,k,v = x @ W{q,k,v}; scores = q k^T / sqrt(H); causal softmax; out = P v.

Design (collective-free, one SPMD-uniform NEFF, no barrier/AllGather/RS):
  - 4 batches x 2 cores per batch. Each core receives the FULL batch x^T
    (bf16, host-pretransposed) and projects Q^T for ALL 4096 queries
    itself; K/V only for the 2048 interleaved key rows it owns
    (128-chunk interleave keeps the causal work perfectly balanced).
  - SPMD uniformity: the host permutes x^T columns per core so the
    core's OWN key chunks sit at even 128-chunk positions. All extraction
    addresses are then identical across cores; causality differences are
    absorbed into per-core 0/1 mask DATA (queries are consistently in the
    permuted order; the host un-permutes the output).
  - Weights are host-reshaped so their DMAs use 1-2KB descriptors (the
    naive [D,128] layout produces 2k+ 256B descriptors that clog the
    queues ahead of the x strips).
  - The PE runs one continuous instruction stream (it only reaches its
    full 2.4GHz p-state when never blocked): Q|K projection quarters,
    V chunks, score chunk-pairs and AV pairs are interleaved so that by
    the time the PE reaches an instruction its inputs are long ready.
    Score pairs are "sprinkled" early between projection quarters to
    start the Scalar engine's exp stream (the 2nd-largest cost, ~40us)
    as soon as possible; exp results go to a big persistent SBUF P
    buffer (72KB/partition), and AV consumes P far behind the exp
    stream, so neither engine ever waits for the other.
  - V is augmented with a ones column so AV also emits the softmax
    denominator. Per-core partial (num^T | den) = [65, 4096] f32 goes
    straight to DRAM; the HOST adds the two partials of each pair,
    divides, and transposes. No on-device collective at all.
"""

import sys

sys.path.insert(0, "/opt/trn_rl_repo")

import numpy as np
import ml_dtypes

B, S, D, H = 4, 4096, 1024, 64
RPC = S // 2            # key rows owned per core
QB = 512                # query block width
NQB = S // QB           # 8 query blocks
NKC = RPC // 128        # 16 local key chunks
BF16 = ml_dtypes.bfloat16

_CACHE = {}


def _build():
    import concourse.bass as bass
    import concourse.mybir as mybir
    from concourse import bacc, tile
    from concourse.bass import ts

    f32 = mybir.dt.float32
    bf16 = mybir.dt.bfloat16
    Alu = mybir.AluOpType
    Act = mybir.ActivationFunctionType

    nc = bacc.Bacc(None, target_bir_lowering=False)

    xt_ext = nc.declare_dram_parameter("xt", [D, S], bf16, isOutput=False)
    # weights pre-shuffled on host: partition p holds all 8 dc-chunks
    wqk_ext = nc.declare_dram_parameter("wqk", [128, 8 * 128], bf16, isOutput=False)
    wv_ext = nc.declare_dram_parameter("wv", [128, 8 * H], bf16, isOutput=False)
    mask_ext = nc.declare_dram_parameter("mask", [128, 1024], bf16, isOutput=False)
    out_ext = nc.declare_dram_parameter("out", [H + 1, S], f32, isOutput=True)

    pairs = [(t, ip) for t in range(NQB) for ip in range(t + 1)]  # 36 chunk-pairs

    with tile.TileContext(nc) as tc:
        with tc.tile_pool(name="persist", bufs=1) as persist:
            # --- persistent SBUF tensors ---
            wqk_sb = persist.tile([128, 8, 128], bf16, tag="wqk")
            wv_sb = persist.tile([128, 8, H], bf16, tag="wv")
            mask_sb = persist.tile([128, 1024], bf16, tag="mask")
            qT = persist.tile([64, S], bf16, tag="qT")
            kT = persist.tile([64, RPC], bf16, tag="kT")
            v_all = persist.tile([128, NKC, H + 1], bf16, tag="v_all")
            p_sb = persist.tile([128, 36, 1024], bf16, tag="p")

            nc.vector.memset(v_all[:, :, H], 1.0)

            nc.sync.dma_start(out=wqk_sb[:], in_=wqk_ext[:])
            nc.scalar.dma_start(out=wv_sb[:], in_=wv_ext[:])
            nc.scalar.dma_start(out=mask_sb[:], in_=mask_ext[:])

            with tc.tile_pool(name="xt", bufs=16) as xt_pool:
                # one tile per (half, dc) strip; 4KB-line DMAs on both
                # hardware queues, half 0 first (the first consumer).
                xts = [[None] * 8 for _ in range(2)]
                for hf in range(2):
                    for dc in range(8):
                        strip = xt_pool.tile(
                            [128, S // 2], bf16, tag="xt", name=f"xt{hf}_{dc}"
                        )
                        xts[hf][dc] = strip
                        eng = nc.sync if dc % 2 == 0 else nc.scalar
                        eng.dma_start(
                            out=strip[:], in_=xt_ext[ts(dc, 128), ts(hf, S // 2)]
                        )

                state = {"st": 0}

                def emit_st_pair(j):
                    t, ip = pairs[j]
                    st2 = st_pool.tile([128, 1024], f32, tag="st", name=f"st{j}")
                    nc.tensor.matmul(
                        st2[:, 0:512],
                        lhsT=kT[:, 256 * ip : 256 * ip + 128],
                        rhs=qT[:, ts(t, QB)],
                        start=True,
                        stop=True,
                        skip_group_check=True,
                    )
                    nc.tensor.matmul(
                        st2[:, 512:1024],
                        lhsT=kT[:, 256 * ip + 128 : 256 * ip + 256],
                        rhs=qT[:, ts(t, QB)],
                        start=True,
                        stop=True,
                        skip_group_check=True,
                    )
                    nc.scalar.activation(p_sb[:, j, :], st2[:], Act.Exp, scale=0.125)
                    if ip == t:  # diagonal pair: multiplicative causal mask
                        nc.vector.tensor_tensor(
                            p_sb[:, j, :], p_sb[:, j, :], mask_sb[:], Alu.mult
                        )

                def emit_st_pairs(n, t_max):
                    while n > 0 and state["st"] < 36 and pairs[state["st"]][0] <= t_max:
                        emit_st_pair(state["st"])
                        state["st"] += 1
                        n -= 1

                av_tiles = {}

                def emit_av_pair(j):
                    t, ip = pairs[j]
                    if ip == 0:
                        av_tiles[t] = av_pool.tile(
                            [H + 1, QB], f32, tag="av", name=f"av{t}"
                        )
                    av = av_tiles[t]
                    nc.tensor.matmul(
                        av[:],
                        lhsT=v_all[:, 2 * ip, :],
                        rhs=p_sb[:, j, 0:512],
                        start=(ip == 0),
                        stop=False,
                        skip_group_check=True,
                    )
                    nc.tensor.matmul(
                        av[:],
                        lhsT=v_all[:, 2 * ip + 1, :],
                        rhs=p_sb[:, j, 512:1024],
                        start=False,
                        stop=(ip == t),
                        skip_group_check=True,
                    )
                    if ip == t:
                        o_sb = o_pool.tile([H + 1, QB], f32, tag="o", name=f"o{t}")
                        nc.vector.tensor_copy(o_sb[:], av[:])
                        nc.sync.dma_start(out=out_ext[:, ts(t, QB)], in_=o_sb[:])

                with tc.tile_pool(name="st", bufs=3, space="PSUM") as st_pool:
                    # --- Q|K projection in sg-pair quarters (dc-outer) ---
                    with tc.tile_pool(name="pj", bufs=2, space="PSUM") as pj_pool:
                        quarters = [(0, (0, 1)), (0, (2, 3)), (1, (4, 5)), (1, (6, 7))]
                        for hf, ss in quarters:
                            qkps = [
                                pj_pool.tile(
                                    [128, QB], f32, tag="qk", name=f"qk{s}"
                                )
                                for s in ss
                            ]
                            for dc in range(8):
                                for x, s in enumerate(ss):
                                    nc.tensor.matmul(
                                        qkps[x][:],
                                        lhsT=wqk_sb[:, dc, :],
                                        rhs=xts[hf][dc][:, ts(s % 4, QB)],
                                        start=(dc == 0),
                                        stop=(dc == 7),
                                        skip_group_check=True,
                                    )
                            # extraction (DVE): Q^T all columns; K^T own chunks
                            # (even 128-chunk positions) of each block.
                            for x, s in enumerate(ss):
                                nc.vector.tensor_copy(
                                    qT[:, ts(s, QB)], qkps[x][0:64, :]
                                )
                                nc.vector.tensor_copy(
                                    kT[:, 256 * s : 256 * s + 128],
                                    qkps[x][64:128, 0:128],
                                )
                                nc.vector.tensor_copy(
                                    kT[:, 256 * s + 128 : 256 * s + 256],
                                    qkps[x][64:128, 256:384],
                                )
                            # feed the Scalar engine's exp stream early
                            emit_st_pairs(3, ss[-1])

                    # --- V chunks (x^T stationary, Wv moving, N=64) ---
                    with tc.tile_pool(name="pv", bufs=2, space="PSUM") as pv_pool:
                        for i in range(NKC):
                            hf, ii = i // 8, i % 8
                            vps = pv_pool.tile([128, H], f32, tag="v", name=f"v{i}")
                            for dc in range(8):
                                nc.tensor.matmul(
                                    vps[:],
                                    lhsT=xts[hf][dc][:, 256 * ii : 256 * ii + 128],
                                    rhs=wv_sb[:, dc, :],
                                    start=(dc == 0),
                                    stop=(dc == 7),
                                )
                            nc.vector.tensor_copy(v_all[:, i, 0:H], vps[:])
                            emit_st_pairs(1, NQB - 1)

                    # --- remaining score pairs + all AV pairs ---
                    with (
                        tc.tile_pool(name="av", bufs=2, space="PSUM") as av_pool,
                        tc.tile_pool(name="o", bufs=2) as o_pool,
                    ):
                        av_ptr = 0
                        while state["st"] < 36:
                            emit_st_pairs(1, NQB - 1)
                            for _ in range(2):
                                if av_ptr < state["st"] - 2:
                                    emit_av_pair(av_ptr)
                                    av_ptr += 1
                        while av_ptr < 36:
                            emit_av_pair(av_ptr)
                            av_ptr += 1

    nc.finalize()
    return nc


def _make_mask2(g: int) -> np.ndarray:
    """[128, 1024] multiplicative mask for the diagonal chunk pair of any
    query block t (t-independent thanks to the per-core permutation).

    Query columns are in permuted order: position pc in the block maps to
    global query chunk offsets delta = [g, 1-g, 2+g, 3-g] (relative to 4t).
    Left half masks own key chunk at global offset g; right half offset 2+g.
    """
    m = np.zeros((128, 1024), dtype=np.float32)
    delta = [g, 1 - g, 2 + g, 3 - g]
    kk = np.arange(128)[:, None]
    qq = np.arange(128)[None, :]
    for half, keyoff in ((0, g), (1, 2 + g)):
        for pc in range(4):
            keep = (128 * (delta[pc] - keyoff) + qq) >= kk
            m[:, half * 512 + pc * 128 : half * 512 + (pc + 1) * 128] = keep
    return m.astype(BF16)


def _swap_pairs(a: np.ndarray) -> np.ndarray:
    """Swap adjacent 128-column chunks (self-inverse permutation)."""
    n = a.shape[-1]
    return np.ascontiguousarray(
        a.reshape(a.shape[:-1] + (n // 256, 2, 128))[..., ::-1, :].reshape(a.shape)
    )


def _shard_inputs(input, Wq, Wk, Wv):
    wqk = np.concatenate([Wq, Wk], axis=1).astype(BF16)       # [1024, 128]
    wv = np.asarray(Wv).astype(BF16)                          # [1024, 64]
    # partition-major reshuffle so the SBUF load uses 1-2KB descriptors:
    # partition p holds [dc, col] for all 8 dc chunks
    wqk_r = np.ascontiguousarray(
        wqk.reshape(8, 128, 128).transpose(1, 0, 2).reshape(128, 8 * 128)
    )
    wv_r = np.ascontiguousarray(
        wv.reshape(8, 128, H).transpose(1, 0, 2).reshape(128, 8 * H)
    )
    masks = [_make_mask2(0), _make_mask2(1)]
    in_maps = []
    for b in range(B):
        xt = np.ascontiguousarray(np.asarray(input)[b].T).astype(BF16)
        xt_sw = _swap_pairs(xt)
        for g in range(2):
            in_maps.append(
                {
                    "xt": xt if g == 0 else xt_sw,
                    "wqk": wqk_r,
                    "wv": wv_r,
                    "mask": masks[g],
                }
            )
    return in_maps


def _unshard(results):
    out = np.empty((B, S, H), dtype=np.float32)
    for b in range(B):
        r0 = results[2 * b]["out"]                      # [65, S] natural order
        r1 = _swap_pairs(results[2 * b + 1]["out"])     # un-permute g=1
        m = r0 + r1
        out[b] = (m[:H] / m[H : H + 1]).T
    return out


def _run(inputs, trace=False):
    from concourse.bass_utils import run_bass_kernel_spmd

    if "nc" not in _CACHE:
        _CACHE["nc"] = _build()
    nc = _CACHE["nc"]
    in_maps = _shard_inputs(**inputs)
    res = run_bass_kernel_spmd(nc, in_maps, core_ids=list(range(8)), trace=trace)
    out = _unshard(res.results)
    return out, res


def kernel(**inputs) -> np.ndarray:
    out, _ = _run(inputs, trace=False)
    return out


# revision 13
# speedup vs baseline: 1.3269x; 1.0743x over previous
"""Distributed causal attention head for TRN2 (8 NeuronCores), v4.

Problem: B=4, S=4096, D=1024, H=64 fp32.
  q,k,v = x @ W{q,k,v}; scores = q k^T / sqrt(H); causal softmax; out = P v.

Design (collective-free, one SPMD-uniform NEFF, no barrier/AllGather/RS):
  - 4 batches x 2 cores per batch. Each core receives the FULL batch x^T
    (bf16, host-pretransposed) and projects Q^T for ALL 4096 queries
    itself; K/V only for the 2048 interleaved key rows it owns
    (128-chunk interleave keeps the causal work perfectly balanced).
  - SPMD uniformity: the host permutes x^T columns per core so the
    core's OWN key chunks sit at even 128-chunk positions. All extraction
    addresses are then identical across cores; causality differences are
    absorbed into per-core 0/1 mask DATA (queries are consistently in the
    permuted order; the host un-permutes the output).
  - Weights are host-reshaped so their DMAs use 1-2KB descriptors (the
    naive [D,128] layout produces 2k+ 256B descriptors that clog the
    queues ahead of the x strips).
  - The PE runs one continuous instruction stream (it only reaches its
    full 2.4GHz p-state when never blocked): Q|K projection quarters,
    V chunks, score chunk-pairs and AV pairs are interleaved so that by
    the time the PE reaches an instruction its inputs are long ready.
    Score pairs are "sprinkled" early between projection quarters to
    start the Scalar engine's exp stream (the 2nd-largest cost, ~40us)
    as soon as possible; exp results go to a big persistent SBUF P
    buffer (72KB/partition), and AV consumes P far behind the exp
    stream, so neither engine ever waits for the other.
  - V is augmented with a ones column so AV also emits the softmax
    denominator. Per-core partial (num^T | den) = [65, 4096] f32 goes
    straight to DRAM; the HOST adds the two partials of each pair,
    divides, and transposes. No on-device collective at all.
"""

import sys

sys.path.insert(0, "/opt/trn_rl_repo")

import numpy as np
import ml_dtypes

B, S, D, H = 4, 4096, 1024, 64
RPC = S // 2            # key rows owned per core
QB = 512                # query block width
NQB = S // QB           # 8 query blocks
NKC = RPC // 128        # 16 local key chunks
BF16 = ml_dtypes.bfloat16

_CACHE = {}


def _build():
    import concourse.bass as bass
    import concourse.mybir as mybir
    from concourse import bacc, tile
    from concourse.bass import ts

    f32 = mybir.dt.float32
    bf16 = mybir.dt.bfloat16
    Alu = mybir.AluOpType
    Act = mybir.ActivationFunctionType

    nc = bacc.Bacc(None, target_bir_lowering=False)

    xt_ext = nc.declare_dram_parameter("xt", [D, S], bf16, isOutput=False)
    # weights pre-shuffled on host: partition p holds all 8 dc-chunks
    wqk_ext = nc.declare_dram_parameter("wqk", [128, 8 * 128], bf16, isOutput=False)
    wv_ext = nc.declare_dram_parameter("wv", [128, 8 * H], bf16, isOutput=False)
    mask_ext = nc.declare_dram_parameter("mask", [128, 1024], bf16, isOutput=False)
    out_ext = nc.declare_dram_parameter("out", [H + 1, S], f32, isOutput=True)

    pairs = [(t, ip) for t in range(NQB) for ip in range(t + 1)]  # 36 chunk-pairs

    with tile.TileContext(nc) as tc:
        with tc.tile_pool(name="persist", bufs=1) as persist:
            # --- persistent SBUF tensors ---
            wqk_sb = persist.tile([128, 8, 128], bf16, tag="wqk")
            wv_sb = persist.tile([128, 8, H], bf16, tag="wv")
            mask_sb = persist.tile([128, 1024], bf16, tag="mask")
            qT = persist.tile([64, S], bf16, tag="qT")
            kT = persist.tile([64, RPC], bf16, tag="kT")
            v_all = persist.tile([128, NKC, H + 1], bf16, tag="v_all")
            p_sb = persist.tile([128, 36, 1024], bf16, tag="p")

            nc.vector.memset(v_all[:, :, H], 1.0)

            # weights+mask at the head of the gpsimd (SWDGE) queue so both
            # hardware queues start streaming x strips immediately
            nc.gpsimd.dma_start(out=wqk_sb[:], in_=wqk_ext[:])
            nc.gpsimd.dma_start(out=wv_sb[:], in_=wv_ext[:])
            nc.gpsimd.dma_start(out=mask_sb[:], in_=mask_ext[:])

            with tc.tile_pool(name="xt", bufs=16) as xt_pool:
                # one tile per (half, dc) strip; 4KB-line DMAs round-robin
                # over all 3 DMA-capable queues, half 0 first.
                dma_q = [nc.sync, nc.scalar, nc.gpsimd]
                xts = [[None] * 8 for _ in range(2)]
                for idx in range(16):
                    hf, dc = idx // 8, idx % 8
                    strip = xt_pool.tile(
                        [128, S // 2], bf16, tag="xt", name=f"xt{hf}_{dc}"
                    )
                    xts[hf][dc] = strip
                    dma_q[idx % 3].dma_start(
                        out=strip[:], in_=xt_ext[ts(dc, 128), ts(hf, S // 2)]
                    )

                state = {"st": 0}

                def emit_st_pair(j):
                    t, ip = pairs[j]
                    st2 = st_pool.tile([128, 1024], f32, tag="st", name=f"st{j}")
                    nc.tensor.matmul(
                        st2[:, 0:512],
                        lhsT=kT[:, 256 * ip : 256 * ip + 128],
                        rhs=qT[:, ts(t, QB)],
                        start=True,
                        stop=True,
                        skip_group_check=True,
                    )
                    nc.tensor.matmul(
                        st2[:, 512:1024],
                        lhsT=kT[:, 256 * ip + 128 : 256 * ip + 256],
                        rhs=qT[:, ts(t, QB)],
                        start=True,
                        stop=True,
                        skip_group_check=True,
                    )
                    nc.scalar.activation(p_sb[:, j, :], st2[:], Act.Exp, scale=0.125)
                    if ip == t:  # diagonal pair: multiplicative causal mask
                        nc.vector.tensor_tensor(
                            p_sb[:, j, :], p_sb[:, j, :], mask_sb[:], Alu.mult
                        )

                def emit_st_pairs(n, t_max):
                    while n > 0 and state["st"] < 36 and pairs[state["st"]][0] <= t_max:
                        emit_st_pair(state["st"])
                        state["st"] += 1
                        n -= 1

                av_tiles = {}

                def emit_av_pair(j):
                    t, ip = pairs[j]
                    if ip == 0:
                        av_tiles[t] = av_pool.tile(
                            [H + 1, QB], f32, tag="av", name=f"av{t}"
                        )
                    av = av_tiles[t]
                    nc.tensor.matmul(
                        av[:],
                        lhsT=v_all[:, 2 * ip, :],
                        rhs=p_sb[:, j, 0:512],
                        start=(ip == 0),
                        stop=False,
                        skip_group_check=True,
                    )
                    nc.tensor.matmul(
                        av[:],
                        lhsT=v_all[:, 2 * ip + 1, :],
                        rhs=p_sb[:, j, 512:1024],
                        start=False,
                        stop=(ip == t),
                        skip_group_check=True,
                    )
                    if ip == t:
                        o_sb = o_pool.tile([H + 1, QB], f32, tag="o", name=f"o{t}")
                        nc.vector.tensor_copy(o_sb[:], av[:])
                        nc.sync.dma_start(out=out_ext[:, ts(t, QB)], in_=o_sb[:])

                with (
                    tc.tile_pool(name="st", bufs=2, space="PSUM") as st_pool,
                    tc.tile_pool(name="av", bufs=2, space="PSUM") as av_pool,
                    tc.tile_pool(name="o", bufs=2) as o_pool,
                ):
                    # --- Q|K projection in sg-pair quarters (dc-outer) ---
                    with tc.tile_pool(name="pj", bufs=2, space="PSUM") as pj_pool:
                        quarters = [(0, (0, 1)), (0, (2, 3)), (1, (4, 5)), (1, (6, 7))]
                        for hf, ss in quarters:
                            qkps = [
                                pj_pool.tile(
                                    [128, QB], f32, tag="qk", name=f"qk{s}"
                                )
                                for s in ss
                            ]
                            for dc in range(8):
                                for x, s in enumerate(ss):
                                    nc.tensor.matmul(
                                        qkps[x][:],
                                        lhsT=wqk_sb[:, dc, :],
                                        rhs=xts[hf][dc][:, ts(s % 4, QB)],
                                        start=(dc == 0),
                                        stop=(dc == 7),
                                        skip_group_check=True,
                                    )
                            # extraction (DVE): Q^T all columns; K^T own chunks
                            # (even 128-chunk positions) of each block.
                            for x, s in enumerate(ss):
                                nc.vector.tensor_copy(
                                    qT[:, ts(s, QB)], qkps[x][0:64, :]
                                )
                                nc.vector.tensor_copy(
                                    kT[:, 256 * s : 256 * s + 128],
                                    qkps[x][64:128, 0:128],
                                )
                                nc.vector.tensor_copy(
                                    kT[:, 256 * s + 128 : 256 * s + 256],
                                    qkps[x][64:128, 256:384],
                                )
                            # feed the Scalar engine's exp stream early
                            emit_st_pairs(3, ss[-1])

                    # --- V chunks (x^T stationary, Wv moving, N=64), with
                    # score pairs and AV pairs drained in between ---
                    av_state = {"av": 0}

                    def drain_avs(n, v_done):
                        while (
                            n > 0
                            and av_state["av"] < state["st"] - 2
                            and 2 * pairs[av_state["av"]][1] + 1 <= v_done
                        ):
                            emit_av_pair(av_state["av"])
                            av_state["av"] += 1
                            n -= 1

                    with tc.tile_pool(name="pv", bufs=2, space="PSUM") as pv_pool:
                        for i in range(NKC):
                            hf, ii = i // 8, i % 8
                            vps = pv_pool.tile([128, H], f32, tag="v", name=f"v{i}")
                            for dc in range(8):
                                nc.tensor.matmul(
                                    vps[:],
                                    lhsT=xts[hf][dc][:, 256 * ii : 256 * ii + 128],
                                    rhs=wv_sb[:, dc, :],
                                    start=(dc == 0),
                                    stop=(dc == 7),
                                )
                            nc.vector.tensor_copy(v_all[:, i, 0:H], vps[:])
                            emit_st_pairs(1, NQB - 1)
                            drain_avs(1, i)

                    # --- remaining score pairs + all AV pairs ---
                    while state["st"] < 36:
                        emit_st_pairs(1, NQB - 1)
                        drain_avs(3, NKC - 1)
                    while av_state["av"] < 36:
                        emit_av_pair(av_state["av"])
                        av_state["av"] += 1

    nc.finalize()
    return nc


def _make_mask2(g: int) -> np.ndarray:
    """[128, 1024] multiplicative mask for the diagonal chunk pair of any
    query block t (t-independent thanks to the per-core permutation).

    Query columns are in permuted order: position pc in the block maps to
    global query chunk offsets delta = [g, 1-g, 2+g, 3-g] (relative to 4t).
    Left half masks own key chunk at global offset g; right half offset 2+g.
    """
    m = np.zeros((128, 1024), dtype=np.float32)
    delta = [g, 1 - g, 2 + g, 3 - g]
    kk = np.arange(128)[:, None]
    qq = np.arange(128)[None, :]
    for half, keyoff in ((0, g), (1, 2 + g)):
        for pc in range(4):
            keep = (128 * (delta[pc] - keyoff) + qq) >= kk
            m[:, half * 512 + pc * 128 : half * 512 + (pc + 1) * 128] = keep
    return m.astype(BF16)


def _swap_pairs(a: np.ndarray) -> np.ndarray:
    """Swap adjacent 128-column chunks (self-inverse permutation)."""
    n = a.shape[-1]
    return np.ascontiguousarray(
        a.reshape(a.shape[:-1] + (n // 256, 2, 128))[..., ::-1, :].reshape(a.shape)
    )


def _shard_inputs(input, Wq, Wk, Wv):
    wqk = np.concatenate([Wq, Wk], axis=1).astype(BF16)       # [1024, 128]
    wv = np.asarray(Wv).astype(BF16)                          # [1024, 64]
    # partition-major reshuffle so the SBUF load uses 1-2KB descriptors:
    # partition p holds [dc, col] for all 8 dc chunks
    wqk_r = np.ascontiguousarray(
        wqk.reshape(8, 128, 128).transpose(1, 0, 2).reshape(128, 8 * 128)
    )
    wv_r = np.ascontiguousarray(
        wv.reshape(8, 128, H).transpose(1, 0, 2).reshape(128, 8 * H)
    )
    masks = [_make_mask2(0), _make_mask2(1)]
    in_maps = []
    for b in range(B):
        xt = np.ascontiguousarray(np.asarray(input)[b].T).astype(BF16)
        xt_sw = _swap_pairs(xt)
        for g in range(2):
            in_maps.append(
                {
                    "xt": xt if g == 0 else xt_sw,
                    "wqk": wqk_r,
                    "wv": wv_r,
                    "mask": masks[g],
                }
            )
    return in_maps


def _unshard(results):
    out = np.empty((B, S, H), dtype=np.float32)
    for b in range(B):
        r0 = results[2 * b]["out"]                      # [65, S] natural order
        r1 = _swap_pairs(results[2 * b + 1]["out"])     # un-permute g=1
        m = r0 + r1
        out[b] = (m[:H] / m[H : H + 1]).T
    return out


def _run(inputs, trace=False):
    from concourse.bass_utils import run_bass_kernel_spmd

    if "nc" not in _CACHE:
        _CACHE["nc"] = _build()
    nc = _CACHE["nc"]
    in_maps = _shard_inputs(**inputs)
    res = run_bass_kernel_spmd(nc, in_maps, core_ids=list(range(8)), trace=trace)
    out = _unshard(res.results)
    return out, res


def kernel(**inputs) -> np.ndarray:
    out, _ = _run(inputs, trace=False)
    return out
